# revision 13
# baseline (speedup 1.0000x reference)
"""Trainium2 Bass kernel for nn_Loc2Cluster (GNN message passing, segment-max).

Computation: agg[c] = elementwise-max over locs with edge to cluster c of
x_locs[loc]; empty clusters -> 0; output = concat([x_clusters, agg], -1).

Strategy (cluster-sharded, zero collectives):
  - Core k owns clusters [4096k, 4096(k+1)).
  - Host routes each edge's loc row to the core owning its dst cluster.
  - Within a core, clusters are sorted by in-degree (desc). Rows are laid
    out in "rounds": round r holds the r-th edge row of every cluster with
    count > r, in sorted-cluster order. Sorted order makes each round a
    contiguous *prefix* of cluster slots, so the whole segment-max becomes
    ~max_degree elementwise tensor_max ops over shrinking prefixes -- no
    data-dependent addressing on device at all.
  - Round block layout is partition-major ([128, M_r/128, 256]) so every
    DMA is a plain contiguous copy and every cluster lives at a fixed
    (partition, chunk) slot of the SBUF accumulator.
  - Round 0 is DMA'd straight into the accumulator (tail slots for empty
    clusters are zero rows -> matches reference's 0-fill, no fixup pass).
  - Output [4096, 512] written per core: left half = x_clusters (sorted),
    right half = accumulator; host unsorts and stacks.
"""

import sys

import numpy as np

if "/opt/trn_rl_repo" not in sys.path:
    sys.path.insert(0, "/opt/trn_rl_repo")

N_LOCS = 262144
N_CLUSTERS = 32768
D = 256
N_CORES = 8
CPC = N_CLUSTERS // N_CORES  # 4096 clusters per core
P = 128
CHUNKS = CPC // P  # 32 chunks of 128 clusters
NEG = np.float32(-1e30)

LAST_RESULTS = None  # BassKernelResults of the most recent run (for profiling)
LAST_NC = None  # compiled Bass module of the most recent run (for TimelineSim)


def _host_prep(x_locs, x_clusters, edge_src, edge_dst):
    """Build per-core round-major row streams + sorted x_clusters shards."""
    x_locs = np.ascontiguousarray(np.asarray(x_locs, dtype=np.float32))
    x_clusters = np.ascontiguousarray(np.asarray(x_clusters, dtype=np.float32))
    src = np.asarray(edge_src).astype(np.int64)
    dst = np.asarray(edge_dst).astype(np.int64)
    n_edges = dst.shape[0]

    counts = np.bincount(dst, minlength=N_CLUSTERS)  # [32768]

    # Global order by count desc, dealt round-robin across cores: cluster
    # with global rank g goes to core g%8 at local rank g//8. This balances
    # the per-core round sizes to within 1 cluster, so the shared (SPMD)
    # round schedule has nearly zero cross-core padding, and each core's
    # local order is automatically count-sorted.
    gorder = np.argsort(-counts, kind="stable")  # [32768] cluster ids by rank
    grank = np.empty_like(gorder)
    grank[gorder] = np.arange(N_CLUSTERS)
    # order[k, s] = cluster id at core k local rank s
    order = np.ascontiguousarray(gorder.reshape(CPC, N_CORES).T)  # [8, CPC]

    # occurrence index of each edge within its dst cluster
    by_dst = np.argsort(dst, kind="stable")
    group_start = np.zeros(N_CLUSTERS, dtype=np.int64)
    np.cumsum(counts[:-1], out=group_start[1:])
    occ = np.empty(n_edges, dtype=np.int64)
    occ[by_dst] = np.arange(n_edges, dtype=np.int64) - group_start[dst[by_dst]]

    g_of = grank[dst]
    core_of = g_of % N_CORES
    rank_of = g_of // N_CORES

    # round schedule: m_r global = #clusters with count > r; per-core max
    # is ceil(m_r/8); round block padded to a multiple of 128 slots
    R = int(counts.max())
    counts_sorted = counts[gorder]
    m_r_g = (counts_sorted[None, :] > np.arange(R)[:, None]).sum(axis=1)
    m_r = (m_r_g + N_CORES - 1) // N_CORES  # per-core max
    M = ((m_r + P - 1) // P) * P
    M[0] = CPC  # round 0 covers every slot (zeros for empty clusters)
    offs = np.zeros(R + 1, dtype=np.int64)
    np.cumsum(M, out=offs[1:])
    TOT = int(offs[-1])

    # slot of each edge inside its core's stream (partition-major blocks)
    X = M // P  # chunks per round
    p_of = rank_of % P
    c_of = rank_of // P
    slot = offs[occ] + p_of * X[occ] + c_of

    slot_src = np.full((N_CORES, TOT), -1, dtype=np.int64)
    slot_src[core_of, slot] = src

    in_maps = []
    for k in range(N_CORES):
        ss = slot_src[k]
        stream = x_locs[np.maximum(ss, 0)]  # [TOT, 256]
        pad = ss < 0
        if pad[:CPC].any():
            stream[np.flatnonzero(pad[:CPC])] = 0.0  # empty clusters -> 0
        padr = np.flatnonzero(pad[CPC:]) + CPC
        if padr.size:
            stream[padr] = NEG  # later-round pads are max-neutral
        xc = x_clusters[order[k]]  # [CPC, D] by sorted rank
        xc = np.ascontiguousarray(
            xc.reshape(CHUNKS, P, D).transpose(1, 0, 2)
        )  # [P, CHUNKS, D]
        in_maps.append({"rows": np.ascontiguousarray(stream), "xc": xc})

    return in_maps, order, M, offs, TOT, x_clusters


def _build_program(R, M, offs, TOT, big_split=8, out_split=4, bufs=5):
    from concourse import bacc, mybir
    from concourse._compat import axon_active
    from concourse.tile import TileContext

    nc = bacc.Bacc(
        "TRN2",
        target_bir_lowering=False,
        debug=not axon_active(),
        num_devices=N_CORES,
    )
    rows_h = nc.dram_tensor("rows", [TOT, D], mybir.dt.float32, kind="ExternalInput")
    xc_h = nc.dram_tensor(
        "xc", [P, CHUNKS, D], mybir.dt.float32, kind="ExternalInput"
    )
    out_h = nc.dram_tensor(
        "out", [P, CHUNKS, 2 * D], mybir.dt.float32, kind="ExternalOutput"
    )

    with TileContext(nc) as tc:
        with (
            tc.tile_pool(name="accp", bufs=1) as accp,
            tc.tile_pool(name="stagep", bufs=bufs) as stagep,
        ):
            acc = accp.tile([P, CHUNKS * D], mybir.dt.float32)
            # round 0: DMA straight into the accumulator, split for
            # DMA-queue parallelism (each split is contiguous in HBM)
            r0 = rows_h.ap()[0:CPC].rearrange("(p x) f -> p (x f)", p=P)
            step = P // big_split
            for q in range(big_split):
                lo, hi = q * step, (q + 1) * step
                nc.sync.dma_start(out=acc[lo:hi, :], in_=r0[lo:hi, :])
            for r in range(1, R):
                Xr = int(M[r]) // P
                w = Xr * D
                blk = rows_h.ap()[int(offs[r]) : int(offs[r]) + int(M[r])].rearrange(
                    "(p x) f -> p (x f)", p=P
                )
                st = stagep.tile([P, CHUNKS * D], mybir.dt.float32, tag="stage")
                nsplit = big_split if Xr >= big_split else (4 if Xr >= 4 else 1)
                step = P // nsplit
                for q in range(nsplit):
                    lo, hi = q * step, (q + 1) * step
                    nc.sync.dma_start(out=st[lo:hi, :w], in_=blk[lo:hi, :])
                nc.vector.tensor_max(
                    out=acc[:, :w], in0=acc[:, :w], in1=st[:, :w]
                )
            # left half of output: x_clusters passthrough (DRAM->DRAM)
            step = P // out_split
            for q in range(out_split):
                lo, hi = q * step, (q + 1) * step
                nc.sync.dma_start(
                    out=out_h.ap()[lo:hi, :, 0:D], in_=xc_h.ap()[lo:hi]
                )
            # right half: the aggregated maxima
            acc3 = acc[:].rearrange("p (x f) -> p x f", f=D)
            for q in range(out_split):
                lo, hi = q * step, (q + 1) * step
                nc.sync.dma_start(
                    out=out_h.ap()[lo:hi, :, D : 2 * D], in_=acc3[lo:hi]
                )
    nc.compile()
    return nc


def kernel(x_locs, x_clusters, edge_src, edge_dst):
    global LAST_RESULTS, LAST_NC
    from concourse.bass_utils import run_bass_kernel_spmd

    in_maps, order, M, offs, TOT, _xc = _host_prep(
        x_locs, x_clusters, edge_src, edge_dst
    )
    R = len(M)
    nc = _build_program(R, M, offs, TOT)
    LAST_NC = nc
    res = run_bass_kernel_spmd(nc, in_maps, list(range(N_CORES)))
    LAST_RESULTS = res

    full = np.empty((N_CLUSTERS, 2 * D), dtype=np.float32)
    for k in range(N_CORES):
        o = np.asarray(res.results[k]["out"])  # [P, CHUNKS, 2D]
        o = o.transpose(1, 0, 2).reshape(CPC, 2 * D)  # indexed by sorted rank
        full[order[k]] = o
    return full
